# revision 1
# baseline (speedup 1.0000x reference)
"""Multi-head attention layer (B=2,S=2048,D=1024,H=16) on 8 TRN2 NeuronCores.

Sharding: data parallel over batch (2) x tensor parallel over heads (4 heads
per core).  Each core computes, for its (batch b, head-group hg):
  QT = (X_b @ Wq[:,cols] + bq + emotion)^T         [256, S]     (n on partitions)
  KT = (Xv_b @ Wk[:,cols] + bk)^T                  [256, Skv]   (compacted keys)
  V  = Xv_b @ Wv[:,cols] + bv                      [Skv, 256]   (natural, +ones col)
  scoresT[j,i] = KT_h-slices x QT_h, exp fused with 1/8 scale + key mask bias,
  OT_h = V_h_aug^T @ PT  (row 64 = softmax denominator l via the ones column),
  out_partial = (OT/l)^T @ Wo[rows,:]              [S, D]
Host compacts the key/value positions by the attention mask (the padding mask
zeroes whole key columns, so invalid positions are dropped before projection),
then sums the 4 partial outputs per batch and adds bo.

Matmuls run in float32r (TRN2 fast-fp32 PE mode, ~1.5e-4 rel accuracy/matmul).
"""
import math
import sys

sys.path.insert(0, "/opt/trn_rl_repo")

import numpy as np

import concourse.bass as bass
import concourse.tile as tile
from concourse import bacc, mybir
from concourse.bass_utils import run_bass_kernel_spmd

B, S, D, H = 2, 2048, 1024, 16
DH = D // H          # 64
HPC = 4              # heads per core
NCOL = HPC * DH      # 256 columns of Wq/Wk/Wv per core
NC2 = NCOL // 128    # 2 partition-chunks of the head dim
ND = D // 128        # 8 contraction chunks
NI = S // 512        # 4 query 512-chunks
NS = S // 128        # 16 query 128-chunks
F32 = mybir.dt.float32
F32R = mybir.dt.float32r
AF = mybir.ActivationFunctionType

_PROGRAM_CACHE = {}


def _chunks(total, step):
    out = []
    o = 0
    while o < total:
        out.append((o, min(step, total - o)))
        o += step
    return out


def build_program(skv: int):
    """One NeuronCore's program; SPMD across 8 cores with different data."""
    nj = skv // 128
    nc = bacc.Bacc("TRN2", target_bir_lowering=False, debug=False, num_devices=8)

    xt = nc.declare_dram_parameter("xt", [D, S], F32R, isOutput=False)
    xtkv = nc.declare_dram_parameter("xtkv", [D, skv], F32R, isOutput=False)
    wq = nc.declare_dram_parameter("wq", [D, NCOL], F32R, isOutput=False)
    wk = nc.declare_dram_parameter("wk", [D, NCOL], F32R, isOutput=False)
    wv = nc.declare_dram_parameter("wv", [D, NCOL], F32R, isOutput=False)
    wo = nc.declare_dram_parameter("wo", [NCOL, D], F32R, isOutput=False)
    bq = nc.declare_dram_parameter("bq", [NCOL], F32, isOutput=False)
    bk = nc.declare_dram_parameter("bk", [NCOL], F32, isOutput=False)
    bv = nc.declare_dram_parameter("bv", [NCOL], F32, isOutput=False)
    ew = nc.declare_dram_parameter("ew", [NCOL], F32, isOutput=False)
    maskb = nc.declare_dram_parameter("maskb", [skv], F32, isOutput=False)
    out = nc.declare_dram_parameter("out", [S, D], F32, isOutput=True)

    with tile.TileContext(nc) as tc:
        with tc.tile_pool(name="singles", bufs=1) as singles:
            # --- persistent SBUF tiles -----------------------------------
            twqa = singles.tile([128, ND * NCOL], F32R, tag="wqa", name="twqa")
            twka = singles.tile([128, ND * NCOL], F32R, tag="wka", name="twka")
            twva = singles.tile([128, ND * NCOL], F32R, tag="wva", name="twva")
            twoa = singles.tile([128, NC2 * D], F32R, tag="woa", name="twoa")
            txkva = singles.tile([128, ND * skv], F32R, tag="xkva", name="txkva")
            twq = [twqa[:, d * NCOL:(d + 1) * NCOL] for d in range(ND)]
            twk = [twka[:, d * NCOL:(d + 1) * NCOL] for d in range(ND)]
            twv = [twva[:, d * NCOL:(d + 1) * NCOL] for d in range(ND)]
            two = [twoa[:, c * D:(c + 1) * D] for c in range(NC2)]
            txkv = [txkva[:, d * skv:(d + 1) * skv] for d in range(ND)]
            tqt = [singles.tile([128, S], F32R, tag=f"qt{c}", name=f"qt{c}") for c in range(NC2)]
            tkt = [singles.tile([128, skv], F32R, tag=f"kt{c}", name=f"kt{c}") for c in range(NC2)]
            tv = [
                [singles.tile([128, DH + 1], F32R, tag=f"v{h}_{j}", name=f"v{h}_{j}") for j in range(nj)]
                for h in range(HPC)
            ]
            tot = [singles.tile([128, S], F32, tag=f"ot{c}", name=f"ot{c}") for c in range(NC2)]
            totn = [singles.tile([128, S], F32R, tag=f"otn{c}", name=f"otn{c}") for c in range(NC2)]
            # softmax denominators: rows 0/32/64/96 hold heads 0..3
            tstage = singles.tile([97, S], F32, tag="lstage", name="tstage")
            trecf = singles.tile([97, S], F32, tag="lrecf", name="trecf")
            trec = singles.tile([97, S], F32R, tag="lrec", name="trec")
            tones4 = singles.tile([97, 64], F32R, tag="ones4", name="tones4")
            tonesf = singles.tile([128, 64], F32, tag="onesf", name="tonesf")
            tmb = [singles.tile([128, 1], F32, tag=f"mb{j}", name=f"mb{j}") for j in range(nj)]
            tbiasq = [singles.tile([128, 1], F32, tag=f"bq{c}", name=f"bq{c}") for c in range(NC2)]
            tbiask = [singles.tile([128, 1], F32, tag=f"bk{c}", name=f"bkt{c}") for c in range(NC2)]
            tbq_raw = [singles.tile([128, 1], F32, tag=f"bqr{c}", name=f"bqr{c}") for c in range(NC2)]
            tew_raw = [singles.tile([128, 1], F32, tag=f"ewr{c}", name=f"ewr{c}") for c in range(NC2)]
            tbvb = singles.tile([128, NCOL], F32, tag="bvb", name="bvb")

            # --- input DMAs ----------------------------------------------
            # sync queue order = critical path order: wq first (Q proj),
            # then xt (streamed inside the Q loop), then wk/xtkv (K proj),
            # wv (V proj), wo (final).  Small tiles go on the gpsimd queue.
            for d in range(ND):
                nc.sync.dma_start(out=twq[d], in_=wq[d * 128:(d + 1) * 128, :])
            for c in range(NC2):
                nc.gpsimd.dma_start(
                    out=tbq_raw[c], in_=bq[c * 128:(c + 1) * 128].unsqueeze(1)
                )
                nc.gpsimd.dma_start(
                    out=tew_raw[c], in_=ew[c * 128:(c + 1) * 128].unsqueeze(1)
                )
                nc.gpsimd.dma_start(
                    out=tbiask[c], in_=bk[c * 128:(c + 1) * 128].unsqueeze(1)
                )
                nc.vector.tensor_add(out=tbiasq[c], in0=tbq_raw[c], in1=tew_raw[c])
            for j in range(nj):
                nc.gpsimd.dma_start(
                    out=tmb[j], in_=maskb[j * 128:(j + 1) * 128].unsqueeze(1)
                )
            bvap = bv.ap()
            bv_bcast = bass.AP(
                tensor=bvap.tensor, offset=bvap.offset, ap=[[0, 128]] + list(bvap.ap)
            )
            nc.gpsimd.dma_start(out=tbvb, in_=bv_bcast)
            # memset cannot write float32r, so round ones through the DVE.
            nc.vector.memset(tonesf, 1.0)
            for h in range(HPC):
                nc.vector.tensor_copy(
                    out=tones4[32 * h:32 * h + 1, :], in_=tonesf[0:1, :]
                )

            # --- phase 1: projections (Q, K, V) --------------------------
            with (
                tc.tile_pool(name="xts", bufs=8) as xts,
                tc.tile_pool(name="pproj", bufs=4, space="PSUM") as pproj,
                tc.tile_pool(name="pv", bufs=2, space="PSUM") as pv,
            ):
                # ones column of V
                for h in range(HPC):
                    for j in range(nj):
                        nc.vector.tensor_copy(
                            out=tv[h][j][:, DH:DH + 1], in_=tonesf[:, 0:1]
                        )
                # QT[n, i] accumulated over d
                for i in range(NI):
                    xt_tiles = []
                    for d in range(ND):
                        xtile = xts.tile([128, 512], F32R, tag="xt", name="xtile")
                        nc.sync.dma_start(
                            out=xtile,
                            in_=xt[d * 128:(d + 1) * 128, i * 512:(i + 1) * 512],
                        )
                        xt_tiles.append(xtile)
                    for c in range(NC2):
                        ps = pproj.tile([128, 512], F32, tag="pp", name="pp")
                        for d in range(ND):
                            nc.tensor.matmul(
                                ps,
                                twq[d][:, c * 128:(c + 1) * 128],
                                xt_tiles[d],
                                start=(d == 0),
                                stop=(d == ND - 1),
                            )
                        nc.vector.tensor_scalar_add(
                            out=tqt[c][:, i * 512:(i + 1) * 512],
                            in0=ps,
                            scalar1=tbiasq[c],
                        )
                # remaining weight/kv loads (behind wq+xt in the sync queue)
                for d in range(ND):
                    nc.sync.dma_start(out=twk[d], in_=wk[d * 128:(d + 1) * 128, :])
                    nc.sync.dma_start(out=txkv[d], in_=xtkv[d * 128:(d + 1) * 128, :])
                for d in range(ND):
                    nc.sync.dma_start(out=twv[d], in_=wv[d * 128:(d + 1) * 128, :])
                for c in range(NC2):
                    nc.sync.dma_start(out=two[c], in_=wo[c * 128:(c + 1) * 128, :])
                # KT[n, j] accumulated over d
                for (jo, jw) in _chunks(skv, 512):
                    for c in range(NC2):
                        ps = pproj.tile([128, 512], F32, tag="pp", name="pp")
                        for d in range(ND):
                            nc.tensor.matmul(
                                ps[:, 0:jw],
                                twk[d][:, c * 128:(c + 1) * 128],
                                txkv[d][:, jo:jo + jw],
                                start=(d == 0),
                                stop=(d == ND - 1),
                            )
                        nc.vector.tensor_scalar_add(
                            out=tkt[c][:, jo:jo + jw],
                            in0=ps[:, 0:jw],
                            scalar1=tbiask[c],
                        )
                # V[j, n] accumulated over d, split per head (+bias bv)
                for j in range(nj):
                    ps = pv.tile([128, NCOL], F32, tag="pv", name="pvt")
                    for d in range(ND):
                        nc.tensor.matmul(
                            ps,
                            txkv[d][:, j * 128:(j + 1) * 128],
                            twv[d],
                            start=(d == 0),
                            stop=(d == ND - 1),
                        )
                    for h in range(HPC):
                        nc.vector.tensor_add(
                            out=tv[h][j][:, 0:DH],
                            in0=ps[:, h * DH:(h + 1) * DH],
                            in1=tbvb[:, h * DH:(h + 1) * DH],
                        )

            # --- phase 2: attention + normalize + output projection ------
            # j-loop software-pipelined: attnV(j-1) is emitted after exp(j),
            # so the in-order PE never head-of-line blocks on the exp; each
            # block's final-projection matmuls are deferred into the next
            # block's j-loop to fill PE bubbles.
            with (
                tc.tile_pool(name="pts", bufs=3) as pts,
                tc.tile_pool(name="obuf", bufs=4) as obuf,
                tc.tile_pool(name="ps2", bufs=2, space="PSUM") as ps2,
                tc.tile_pool(name="pot", bufs=2, space="PSUM") as pot,
                tc.tile_pool(name="plf", bufs=2, space="PSUM") as plf,
            ):
                pending = []

                def emit_pf(i, so):
                    sidx = i * 4 + so
                    ssl = slice(sidx * 128, (sidx + 1) * 128)
                    for n in range(2):
                        nsl = slice(n * 512, (n + 1) * 512)
                        pf = plf.tile([128, 512], F32, tag="plf", name="pft")
                        for c in range(NC2):
                            nc.tensor.matmul(
                                pf,
                                totn[c][:, ssl],
                                two[c][:, nsl],
                                start=(c == 0),
                                stop=(c == NC2 - 1),
                            )
                        ob = obuf.tile([128, 512], F32, tag="ob", name="obt")
                        nc.vector.tensor_copy(out=ob, in_=pf)
                        nc.sync.dma_start(out=out[ssl, nsl], in_=ob)

                for i in range(NI):
                    isl = slice(i * 512, (i + 1) * 512)
                    for c in range(NC2):
                        hA, hB = 2 * c, 2 * c + 1
                        potA = pot.tile([DH + 1, 512], F32, tag="pot", name="pott")
                        potB = pot.tile([DH + 1, 512], F32, tag="pot", name="pott")
                        pts_hist = []
                        for j in range(nj):
                            pscore = ps2.tile([128, 1024], F32, tag="ps", name="pscore")
                            nc.tensor.matmul(
                                pscore[:, 0:512],
                                tkt[c][0:64, j * 128:(j + 1) * 128],
                                tqt[c][0:64, isl],
                                start=True,
                                stop=True,
                                tile_position=(0, 0),
                            )
                            nc.tensor.matmul(
                                pscore[:, 512:1024],
                                tkt[c][64:128, j * 128:(j + 1) * 128],
                                tqt[c][64:128, isl],
                                start=True,
                                stop=True,
                                tile_position=(64, 0),
                            )
                            pt = pts.tile([128, 1024], F32R, tag="pt", name="ptile")
                            nc.scalar.activation(
                                out=pt,
                                in_=pscore,
                                func=AF.Exp,
                                bias=tmb[j],
                                scale=1.0 / math.sqrt(DH),
                            )
                            pts_hist.append(pt)
                            if j > 0:
                                pprev = pts_hist[j - 1]
                                nc.tensor.matmul(
                                    potA, tv[hA][j - 1], pprev[:, 0:512],
                                    start=(j - 1 == 0), stop=False,
                                )
                                nc.tensor.matmul(
                                    potB, tv[hB][j - 1], pprev[:, 512:1024],
                                    start=(j - 1 == 0), stop=False,
                                )
                            if pending and (j % 2 == 1 or len(pending) > 6):
                                pending.pop(0)()
                        nc.tensor.matmul(
                            potA, tv[hA][nj - 1], pts_hist[nj - 1][:, 0:512],
                            start=(nj == 1), stop=True,
                        )
                        nc.tensor.matmul(
                            potB, tv[hB][nj - 1], pts_hist[nj - 1][:, 512:1024],
                            start=(nj == 1), stop=True,
                        )
                        nc.vector.tensor_copy(out=tot[c][0:64, isl], in_=potA[0:DH, :])
                        nc.vector.tensor_copy(out=tot[c][64:128, isl], in_=potB[0:DH, :])
                        nc.vector.tensor_copy(
                            out=tstage[32 * hA:32 * hA + 1, isl],
                            in_=potA[DH:DH + 1, :],
                        )
                        nc.vector.tensor_copy(
                            out=tstage[32 * hB:32 * hB + 1, isl],
                            in_=potB[DH:DH + 1, :],
                        )
                    # batched softmax-denominator reciprocal for this block
                    nc.vector.reciprocal_approx_fast(
                        out=trecf[:, isl], in_=tstage[:, isl]
                    )
                    nc.vector.tensor_copy(out=trec[:, isl], in_=trecf[:, isl])
                    for c in range(NC2):
                        hA, hB = 2 * c, 2 * c + 1
                        plA = plf.tile([64, 512], F32, tag="plf", name="plA")
                        plB = plf.tile([64, 512], F32, tag="plf", name="plB")
                        nc.tensor.matmul(
                            plA,
                            tones4[32 * hA:32 * hA + 1, :],
                            trec[32 * hA:32 * hA + 1, isl],
                            start=True,
                            stop=True,
                            tile_position=(32 * hA, 0),
                        )
                        nc.tensor.matmul(
                            plB,
                            tones4[32 * hB:32 * hB + 1, :],
                            trec[32 * hB:32 * hB + 1, isl],
                            start=True,
                            stop=True,
                            tile_position=(32 * hB, 0),
                        )
                        nc.vector.tensor_mul(
                            out=totn[c][0:64, isl], in0=tot[c][0:64, isl], in1=plA
                        )
                        nc.vector.tensor_mul(
                            out=totn[c][64:128, isl], in0=tot[c][64:128, isl], in1=plB
                        )
                    for so in range(4):
                        pending.append(lambda i=i, so=so: emit_pf(i, so))
                while pending:
                    pending.pop(0)()

    nc.compile()
    return nc


def _get_program(skv):
    if skv not in _PROGRAM_CACHE:
        _PROGRAM_CACHE[skv] = build_program(skv)
    return _PROGRAM_CACHE[skv]


def _shard_inputs(hidden_states, attention_mask, Wq, bq, Wk, bk, Wv, bv,
                  emotion_w, Wo, bo):
    hs = np.asarray(hidden_states, dtype=np.float32)
    mask = np.asarray(attention_mask)
    Wq = np.asarray(Wq, dtype=np.float32)
    Wk = np.asarray(Wk, dtype=np.float32)
    Wv = np.asarray(Wv, dtype=np.float32)
    Wo = np.asarray(Wo, dtype=np.float32)
    bq = np.asarray(bq, dtype=np.float32)
    bk = np.asarray(bk, dtype=np.float32)
    bv = np.asarray(bv, dtype=np.float32)
    ew = np.asarray(emotion_w, dtype=np.float32)

    idx = [np.nonzero(mask[b])[0] for b in range(B)]
    sv = max(len(ix) for ix in idx)
    skv = max(128, ((sv + 127) // 128) * 128)

    in_maps = []
    for b in range(B):
        xt_b = np.ascontiguousarray(hs[b].T)  # [D, S]
        xtkv_b = np.zeros((D, skv), dtype=np.float32)
        xtkv_b[:, : len(idx[b])] = hs[b][idx[b]].T
        maskb_b = np.zeros(skv, dtype=np.float32)
        maskb_b[len(idx[b]):] = -1e30
        for hg in range(H // HPC):
            cols = slice(hg * NCOL, (hg + 1) * NCOL)
            in_maps.append(
                {
                    "xt": xt_b,
                    "xtkv": xtkv_b,
                    "wq": np.ascontiguousarray(Wq[:, cols]),
                    "wk": np.ascontiguousarray(Wk[:, cols]),
                    "wv": np.ascontiguousarray(Wv[:, cols]),
                    "wo": np.ascontiguousarray(Wo[cols, :]),
                    "bq": np.ascontiguousarray(bq[cols]),
                    "bk": np.ascontiguousarray(bk[cols]),
                    "bv": np.ascontiguousarray(bv[cols]),
                    "ew": np.ascontiguousarray(
                        ew[hg * HPC:(hg + 1) * HPC].reshape(NCOL)
                    ),
                    "maskb": maskb_b,
                }
            )
    return in_maps, skv, np.asarray(bo, dtype=np.float32)


def run(inputs, trace=False, trace_kwargs=None):
    in_maps, skv, bo = _shard_inputs(**inputs)
    nc = _get_program(skv)
    res = run_bass_kernel_spmd(
        nc,
        in_maps,
        core_ids=list(range(8)),
        trace=trace,
        **(trace_kwargs or {}),
    )
    out = np.zeros((B, S, D), dtype=np.float32)
    for b in range(B):
        acc = np.zeros((S, D), dtype=np.float64)
        for hg in range(4):
            acc += res.results[b * 4 + hg]["out"]
        out[b] = (acc + bo).astype(np.float32)
    return out, res


def kernel(**inputs):
    out, _ = run(inputs, trace=False)
    return out



# revision 11
# speedup vs baseline: 1.1580x; 1.1580x over previous
"""Multi-head attention layer (B=2,S=2048,D=1024,H=16) on 8 TRN2 NeuronCores.

Sharding: data parallel over batch (2) x tensor parallel over heads (4 heads
per core).  Each core computes, for its (batch b, head-group hg):
  QT = (X_b @ Wq[:,cols] + bq + emotion)^T         [256, S]     (n on partitions)
  KT = (Xv_b @ Wk[:,cols] + bk)^T                  [256, Skv]   (compacted keys)
  V  = Xv_b @ Wv[:,cols] + bv                      [Skv, 256]   (natural, +ones col)
  scoresT[j,i] = KT_h-slices x QT_h, exp fused with 1/8 scale + key mask bias,
  OT_h = V_h_aug^T @ PT  (row 64 = softmax denominator l via the ones column),
  out_partial = (OT/l)^T @ Wo[rows,:]              [S, D]
Host compacts the key/value positions by the attention mask (the padding mask
zeroes whole key columns, so invalid positions are dropped before projection),
then sums the 4 partial outputs per batch and adds bo.

Matmuls run in bfloat16 with fp32 PSUM accumulation: on TRN2 hardware a
bf16 matmul streams 1 row/cycle vs fp32r's 2, and avoids the fp32
high-power duty-cycle throttle.  Measured end-to-end rel err ~6e-3 vs
the fp32 reference (tolerance 2e-2).
"""
import math
import sys

sys.path.insert(0, "/opt/trn_rl_repo")

import numpy as np
import ml_dtypes

import concourse.bass as bass
import concourse.tile as tile
from concourse import bacc, mybir
from concourse.bass_utils import run_bass_kernel_spmd

B, S, D, H = 2, 2048, 1024, 16
DH = D // H          # 64
HPC = 4              # heads per core
NCOL = HPC * DH      # 256 columns of Wq/Wk/Wv per core
NC2 = NCOL // 128    # 2 partition-chunks of the head dim
ND = D // 128        # 8 contraction chunks
NI = S // 512        # 4 query 512-chunks
NS = S // 128        # 16 query 128-chunks
F32 = mybir.dt.float32
BF16 = mybir.dt.bfloat16
AF = mybir.ActivationFunctionType

BF16NP = ml_dtypes.bfloat16

_PROGRAM_CACHE = {}


def _chunks(total, step):
    out = []
    o = 0
    while o < total:
        out.append((o, min(step, total - o)))
        o += step
    return out


def build_program(skv: int):
    """One NeuronCore's program; SPMD across 8 cores with different data."""
    nj = skv // 128
    nc = bacc.Bacc("TRN2", target_bir_lowering=False, debug=False, num_devices=8)

    xt = nc.declare_dram_parameter("xt", [D, S], BF16, isOutput=False)
    xtkv = nc.declare_dram_parameter("xtkv", [D, skv], BF16, isOutput=False)
    wq = nc.declare_dram_parameter("wq", [D, NCOL], BF16, isOutput=False)
    wk = nc.declare_dram_parameter("wk", [D, NCOL], BF16, isOutput=False)
    wv = nc.declare_dram_parameter("wv", [D, NCOL], BF16, isOutput=False)
    wo = nc.declare_dram_parameter("wo", [NCOL, D], BF16, isOutput=False)
    bq = nc.declare_dram_parameter("bq", [NCOL], F32, isOutput=False)
    bk = nc.declare_dram_parameter("bk", [NCOL], F32, isOutput=False)
    bv = nc.declare_dram_parameter("bv", [NCOL], F32, isOutput=False)
    ew = nc.declare_dram_parameter("ew", [NCOL], F32, isOutput=False)
    maskb = nc.declare_dram_parameter("maskb", [skv], F32, isOutput=False)
    out = nc.declare_dram_parameter("out", [S, D], BF16, isOutput=True)

    with tile.TileContext(nc) as tc:
        with tc.tile_pool(name="singles", bufs=1) as singles:
            # --- persistent SBUF tiles -----------------------------------
            twqa = singles.tile([128, ND * NCOL], BF16, tag="wqa", name="twqa")
            twka = singles.tile([128, ND * NCOL], BF16, tag="wka", name="twka")
            twva = singles.tile([128, ND * NCOL], BF16, tag="wva", name="twva")
            twoa = singles.tile([128, NC2 * D], BF16, tag="woa", name="twoa")
            txkva = singles.tile([128, ND * skv], BF16, tag="xkva", name="txkva")
            twq = [twqa[:, d * NCOL:(d + 1) * NCOL] for d in range(ND)]
            twk = [twka[:, d * NCOL:(d + 1) * NCOL] for d in range(ND)]
            twv = [twva[:, d * NCOL:(d + 1) * NCOL] for d in range(ND)]
            two = [twoa[:, c * D:(c + 1) * D] for c in range(NC2)]
            txkv = [txkva[:, d * skv:(d + 1) * skv] for d in range(ND)]
            tqt = [singles.tile([128, S], BF16, tag=f"qt{c}", name=f"qt{c}") for c in range(NC2)]
            tkt = [singles.tile([128, skv], BF16, tag=f"kt{c}", name=f"kt{c}") for c in range(NC2)]
            tv = [
                [singles.tile([128, DH + 1], BF16, tag=f"v{h}_{j}", name=f"v{h}_{j}") for j in range(nj)]
                for h in range(HPC)
            ]
            tot = [singles.tile([128, S], F32, tag=f"ot{c}", name=f"ot{c}") for c in range(NC2)]
            totn = [singles.tile([128, S], BF16, tag=f"otn{c}", name=f"otn{c}") for c in range(NC2)]
            # softmax denominators: rows 0/32/64/96 hold heads 0..3
            tstage = singles.tile([97, S], F32, tag="lstage", name="tstage")
            trecf = singles.tile([97, S], F32, tag="lrecf", name="trecf")
            trec = singles.tile([97, S], BF16, tag="lrec", name="trec")
            tones4 = singles.tile([97, 64], BF16, tag="ones4", name="tones4")
            tonesf = singles.tile([128, 64], F32, tag="onesf", name="tonesf")
            tmb = [singles.tile([128, 1], F32, tag=f"mb{j}", name=f"mb{j}") for j in range(nj)]
            tbiasq = [singles.tile([128, 1], F32, tag=f"bq{c}", name=f"bq{c}") for c in range(NC2)]
            tbiask = [singles.tile([128, 1], F32, tag=f"bk{c}", name=f"bkt{c}") for c in range(NC2)]
            tbq_raw = [singles.tile([128, 1], F32, tag=f"bqr{c}", name=f"bqr{c}") for c in range(NC2)]
            tew_raw = [singles.tile([128, 1], F32, tag=f"ewr{c}", name=f"ewr{c}") for c in range(NC2)]
            tbvb = singles.tile([128, NCOL], F32, tag="bvb", name="bvb")

            # --- input DMAs ----------------------------------------------
            # sync queue order = critical path order: wq first (Q proj),
            # then xt (streamed inside the Q loop), then wk/xtkv (K proj),
            # wv (V proj), wo (final).  Small tiles go on the gpsimd queue.
            for d in range(ND):
                nc.sync.dma_start(out=twq[d], in_=wq[d * 128:(d + 1) * 128, :])
            for c in range(NC2):
                nc.gpsimd.dma_start(
                    out=tbq_raw[c], in_=bq[c * 128:(c + 1) * 128].unsqueeze(1)
                )
                nc.gpsimd.dma_start(
                    out=tew_raw[c], in_=ew[c * 128:(c + 1) * 128].unsqueeze(1)
                )
                nc.gpsimd.dma_start(
                    out=tbiask[c], in_=bk[c * 128:(c + 1) * 128].unsqueeze(1)
                )
                nc.vector.tensor_add(out=tbiasq[c], in0=tbq_raw[c], in1=tew_raw[c])
            for j in range(nj):
                nc.gpsimd.dma_start(
                    out=tmb[j], in_=maskb[j * 128:(j + 1) * 128].unsqueeze(1)
                )
            bvap = bv.ap()
            bv_bcast = bass.AP(
                tensor=bvap.tensor, offset=bvap.offset, ap=[[0, 128]] + list(bvap.ap)
            )
            nc.gpsimd.dma_start(out=tbvb, in_=bv_bcast)
            # memset cannot write float32r, so round ones through the DVE.
            nc.vector.memset(tonesf, 1.0)
            for h in range(HPC):
                nc.vector.tensor_copy(
                    out=tones4[32 * h:32 * h + 1, :], in_=tonesf[0:1, :]
                )

            # --- phase 1: projections (Q, K, V) --------------------------
            with (
                tc.tile_pool(name="xts", bufs=8) as xts,
                tc.tile_pool(name="pproj", bufs=4, space="PSUM") as pproj,
                tc.tile_pool(name="pv", bufs=2, space="PSUM") as pv,
            ):
                # ones column of V
                for h in range(HPC):
                    for j in range(nj):
                        nc.vector.tensor_copy(
                            out=tv[h][j][:, DH:DH + 1], in_=tonesf[:, 0:1]
                        )
                # QT[n, i] accumulated over d
                for i in range(NI):
                    xt_tiles = []
                    for d in range(ND):
                        xtile = xts.tile([128, 512], BF16, tag="xt", name="xtile")
                        nc.sync.dma_start(
                            out=xtile,
                            in_=xt[d * 128:(d + 1) * 128, i * 512:(i + 1) * 512],
                        )
                        xt_tiles.append(xtile)
                    for c in range(NC2):
                        ps = pproj.tile([128, 512], F32, tag="pp", name="pp")
                        for d in range(ND):
                            nc.tensor.matmul(
                                ps,
                                twq[d][:, c * 128:(c + 1) * 128],
                                xt_tiles[d],
                                start=(d == 0),
                                stop=(d == ND - 1),
                            )
                        nc.vector.tensor_scalar_add(
                            out=tqt[c][:, i * 512:(i + 1) * 512],
                            in0=ps,
                            scalar1=tbiasq[c],
                        )
                # remaining weight/kv loads (behind wq+xt in the sync queue)
                for d in range(ND):
                    nc.sync.dma_start(out=twk[d], in_=wk[d * 128:(d + 1) * 128, :])
                    nc.sync.dma_start(out=txkv[d], in_=xtkv[d * 128:(d + 1) * 128, :])
                for d in range(ND):
                    nc.sync.dma_start(out=twv[d], in_=wv[d * 128:(d + 1) * 128, :])
                for c in range(NC2):
                    nc.sync.dma_start(out=two[c], in_=wo[c * 128:(c + 1) * 128, :])
                # KT[n, j] accumulated over d
                for (jo, jw) in _chunks(skv, 512):
                    for c in range(NC2):
                        ps = pproj.tile([128, 512], F32, tag="pp", name="pp")
                        for d in range(ND):
                            nc.tensor.matmul(
                                ps[:, 0:jw],
                                twk[d][:, c * 128:(c + 1) * 128],
                                txkv[d][:, jo:jo + jw],
                                start=(d == 0),
                                stop=(d == ND - 1),
                            )
                        nc.vector.tensor_scalar_add(
                            out=tkt[c][:, jo:jo + jw],
                            in0=ps[:, 0:jw],
                            scalar1=tbiask[c],
                        )
                # V[j, n] accumulated over d, split per head (+bias bv)
                for j in range(nj):
                    ps = pv.tile([128, NCOL], F32, tag="pv", name="pvt")
                    for d in range(ND):
                        nc.tensor.matmul(
                            ps,
                            txkv[d][:, j * 128:(j + 1) * 128],
                            twv[d],
                            start=(d == 0),
                            stop=(d == ND - 1),
                        )
                    for h in range(HPC):
                        nc.vector.tensor_add(
                            out=tv[h][j][:, 0:DH],
                            in0=ps[:, h * DH:(h + 1) * DH],
                            in1=tbvb[:, h * DH:(h + 1) * DH],
                        )

            # --- phase 2: attention + normalize + output projection ------
            # j-loop software-pipelined: attnV(j-1) is emitted after exp(j),
            # so the in-order PE never head-of-line blocks on the exp; each
            # block's final-projection matmuls are deferred into the next
            # block's j-loop to fill PE bubbles.
            with (
                tc.tile_pool(name="pts", bufs=3) as pts,
                tc.tile_pool(name="obuf", bufs=4) as obuf,
                tc.tile_pool(name="ps2", bufs=2, space="PSUM") as ps2,
                tc.tile_pool(name="pot", bufs=2, space="PSUM") as pot,
                tc.tile_pool(name="plf", bufs=2, space="PSUM") as plf,
            ):
                pending = []

                def emit_pf(i, so):
                    sidx = i * 4 + so
                    ssl = slice(sidx * 128, (sidx + 1) * 128)
                    for n in range(2):
                        nsl = slice(n * 512, (n + 1) * 512)
                        pf = plf.tile([128, 512], F32, tag="plf", name="pft")
                        for c in range(NC2):
                            nc.tensor.matmul(
                                pf,
                                totn[c][:, ssl],
                                two[c][:, nsl],
                                start=(c == 0),
                                stop=(c == NC2 - 1),
                            )
                        ob = obuf.tile([128, 512], BF16, tag="ob", name="obt")
                        nc.vector.tensor_copy(out=ob, in_=pf)
                        nc.sync.dma_start(out=out[ssl, nsl], in_=ob)

                for i in range(NI):
                    isl = slice(i * 512, (i + 1) * 512)
                    for c in range(NC2):
                        hA, hB = 2 * c, 2 * c + 1
                        potA = pot.tile([DH + 1, 512], F32, tag="pot", name="pott")
                        potB = pot.tile([DH + 1, 512], F32, tag="pot", name="pott")
                        pts_hist = []
                        for j in range(nj):
                            pscore = ps2.tile([128, 1024], F32, tag="ps", name="pscore")
                            nc.tensor.matmul(
                                pscore[:, 0:512],
                                tkt[c][0:64, j * 128:(j + 1) * 128],
                                tqt[c][0:64, isl],
                                start=True,
                                stop=True,
                                tile_position=(0, 0),
                            )
                            nc.tensor.matmul(
                                pscore[:, 512:1024],
                                tkt[c][64:128, j * 128:(j + 1) * 128],
                                tqt[c][64:128, isl],
                                start=True,
                                stop=True,
                                tile_position=(64, 0),
                            )
                            pt = pts.tile([128, 1024], BF16, tag="pt", name="ptile")
                            nc.scalar.activation(
                                out=pt,
                                in_=pscore,
                                func=AF.Exp,
                                bias=tmb[j],
                                scale=1.0 / math.sqrt(DH),
                            )
                            pts_hist.append(pt)
                            if j > 0:
                                pprev = pts_hist[j - 1]
                                nc.tensor.matmul(
                                    potA, tv[hA][j - 1], pprev[:, 0:512],
                                    start=(j - 1 == 0), stop=False,
                                )
                                nc.tensor.matmul(
                                    potB, tv[hB][j - 1], pprev[:, 512:1024],
                                    start=(j - 1 == 0), stop=False,
                                )
                            if pending and (j % 2 == 1 or len(pending) > 6):
                                pending.pop(0)()
                        nc.tensor.matmul(
                            potA, tv[hA][nj - 1], pts_hist[nj - 1][:, 0:512],
                            start=(nj == 1), stop=True,
                        )
                        nc.tensor.matmul(
                            potB, tv[hB][nj - 1], pts_hist[nj - 1][:, 512:1024],
                            start=(nj == 1), stop=True,
                        )
                        nc.vector.tensor_copy(out=tot[c][0:64, isl], in_=potA[0:DH, :])
                        nc.vector.tensor_copy(out=tot[c][64:128, isl], in_=potB[0:DH, :])
                        nc.vector.tensor_copy(
                            out=tstage[32 * hA:32 * hA + 1, isl],
                            in_=potA[DH:DH + 1, :],
                        )
                        nc.vector.tensor_copy(
                            out=tstage[32 * hB:32 * hB + 1, isl],
                            in_=potB[DH:DH + 1, :],
                        )
                    # batched softmax-denominator reciprocal for this block
                    nc.vector.reciprocal_approx_fast(
                        out=trecf[:, isl], in_=tstage[:, isl]
                    )
                    nc.vector.tensor_copy(out=trec[:, isl], in_=trecf[:, isl])
                    for c in range(NC2):
                        hA, hB = 2 * c, 2 * c + 1
                        plA = plf.tile([64, 512], F32, tag="plf", name="plA")
                        plB = plf.tile([64, 512], F32, tag="plf", name="plB")
                        nc.tensor.matmul(
                            plA,
                            tones4[32 * hA:32 * hA + 1, :],
                            trec[32 * hA:32 * hA + 1, isl],
                            start=True,
                            stop=True,
                            tile_position=(32 * hA, 0),
                        )
                        nc.tensor.matmul(
                            plB,
                            tones4[32 * hB:32 * hB + 1, :],
                            trec[32 * hB:32 * hB + 1, isl],
                            start=True,
                            stop=True,
                            tile_position=(32 * hB, 0),
                        )
                        nc.vector.tensor_mul(
                            out=totn[c][0:64, isl], in0=tot[c][0:64, isl], in1=plA
                        )
                        nc.vector.tensor_mul(
                            out=totn[c][64:128, isl], in0=tot[c][64:128, isl], in1=plB
                        )
                    for so in range(4):
                        pending.append(lambda i=i, so=so: emit_pf(i, so))
                while pending:
                    pending.pop(0)()

    nc.compile()
    return nc


def _get_program(skv):
    if skv not in _PROGRAM_CACHE:
        _PROGRAM_CACHE[skv] = build_program(skv)
    return _PROGRAM_CACHE[skv]


def _shard_inputs(hidden_states, attention_mask, Wq, bq, Wk, bk, Wv, bv,
                  emotion_w, Wo, bo):
    hs = np.asarray(hidden_states, dtype=np.float32)
    mask = np.asarray(attention_mask)
    Wq = np.asarray(Wq, dtype=np.float32)
    Wk = np.asarray(Wk, dtype=np.float32)
    Wv = np.asarray(Wv, dtype=np.float32)
    Wo = np.asarray(Wo, dtype=np.float32)
    bq = np.asarray(bq, dtype=np.float32)
    bk = np.asarray(bk, dtype=np.float32)
    bv = np.asarray(bv, dtype=np.float32)
    ew = np.asarray(emotion_w, dtype=np.float32)

    idx = [np.nonzero(mask[b])[0] for b in range(B)]
    sv = max(len(ix) for ix in idx)
    skv = max(128, ((sv + 127) // 128) * 128)

    in_maps = []
    for b in range(B):
        xt_b = np.ascontiguousarray(hs[b].T).astype(BF16NP)  # [D, S]
        xtkv_b = np.zeros((D, skv), dtype=BF16NP)
        xtkv_b[:, : len(idx[b])] = hs[b][idx[b]].T.astype(BF16NP)
        maskb_b = np.zeros(skv, dtype=np.float32)
        maskb_b[len(idx[b]):] = -1e30
        for hg in range(H // HPC):
            cols = slice(hg * NCOL, (hg + 1) * NCOL)
            in_maps.append(
                {
                    "xt": xt_b,
                    "xtkv": xtkv_b,
                    "wq": np.ascontiguousarray(Wq[:, cols]).astype(BF16NP),
                    "wk": np.ascontiguousarray(Wk[:, cols]).astype(BF16NP),
                    "wv": np.ascontiguousarray(Wv[:, cols]).astype(BF16NP),
                    "wo": np.ascontiguousarray(Wo[cols, :]).astype(BF16NP),
                    "bq": np.ascontiguousarray(bq[cols]),
                    "bk": np.ascontiguousarray(bk[cols]),
                    "bv": np.ascontiguousarray(bv[cols]),
                    "ew": np.ascontiguousarray(
                        ew[hg * HPC:(hg + 1) * HPC].reshape(NCOL)
                    ),
                    "maskb": maskb_b,
                }
            )
    return in_maps, skv, np.asarray(bo, dtype=np.float32)


def run(inputs, trace=False, trace_kwargs=None):
    in_maps, skv, bo = _shard_inputs(**inputs)
    nc = _get_program(skv)
    res = run_bass_kernel_spmd(
        nc,
        in_maps,
        core_ids=list(range(8)),
        trace=trace,
        **(trace_kwargs or {}),
    )
    out = np.zeros((B, S, D), dtype=np.float32)
    for b in range(B):
        acc = np.zeros((S, D), dtype=np.float64)
        for hg in range(4):
            acc += np.asarray(res.results[b * 4 + hg]["out"], dtype=np.float32)
        out[b] = (acc + bo).astype(np.float32)
    return out, res


def kernel(**inputs):
    out, _ = run(inputs, trace=False)
    return out



# revision 20
# speedup vs baseline: 1.2593x; 1.0875x over previous
"""Multi-head attention layer (B=2,S=2048,D=1024,H=16) on 8 TRN2 NeuronCores.

Sharding: data parallel over batch (2) x tensor parallel over heads (4 heads
per core).  Each core computes, for its (batch b, head-group hg):
  QT = (X_b @ Wq[:,cols] + bq + emotion)^T         [256, S]     (n on partitions)
  KT = (Xv_b @ Wk[:,cols] + bk)^T                  [256, Skv]   (compacted keys)
  V  = Xv_b @ Wv[:,cols] + bv                      [Skv, 256]   (natural, +ones col)
  scoresT[j,i] = KT_h-slices x QT_h, exp fused with 1/8 scale + key mask bias,
  OT_h = V_h_aug^T @ PT  (row 64 = softmax denominator l via the ones column),
  out_partial = (OT/l)^T @ Wo[rows,:]              [S, D]
Host compacts the key/value positions by the attention mask (the padding mask
zeroes whole key columns, so invalid positions are dropped before projection),
then sums the 4 partial outputs per batch and adds bo.

Matmuls run in bfloat16 with fp32 PSUM accumulation: on TRN2 hardware a
bf16 matmul streams 1 row/cycle vs fp32r's 2, and avoids the fp32
high-power duty-cycle throttle.  Measured end-to-end rel err ~7e-3 vs
the fp32 reference (tolerance 2e-2).

Big tensors are host-packed into [128, n*cols] chunk-major layout so every
DMA moves 4KB-contiguous lines, split across the three DGE queues (sync,
scalar, gpsimd) to keep phase-1 loads off each other's critical path.
"""
import math
import sys

sys.path.insert(0, "/opt/trn_rl_repo")

import numpy as np
import ml_dtypes

import concourse.bass as bass
import concourse.tile as tile
from concourse import bacc, mybir
from concourse.bass_utils import run_bass_kernel_spmd

B, S, D, H = 2, 2048, 1024, 16
DH = D // H          # 64
HPC = 4              # heads per core
NCOL = HPC * DH      # 256 columns of Wq/Wk/Wv per core
NC2 = NCOL // 128    # 2 partition-chunks of the head dim
ND = D // 128        # 8 contraction chunks
NI = S // 512        # 4 query 512-chunks
NS = S // 128        # 16 query 128-chunks
F32 = mybir.dt.float32
BF16 = mybir.dt.bfloat16
AF = mybir.ActivationFunctionType

BF16NP = ml_dtypes.bfloat16

_PROGRAM_CACHE = {}


def _chunks(total, step):
    out = []
    o = 0
    while o < total:
        out.append((o, min(step, total - o)))
        o += step
    return out


def build_program(skv: int):
    """One NeuronCore's program; SPMD across 8 cores with different data."""
    nj = skv // 128
    nc = bacc.Bacc("TRN2", target_bir_lowering=False, debug=False, num_devices=8)

    # chunk-major packed layouts: [:, k*cols:(k+1)*cols] is partition-chunk k
    xt = nc.declare_dram_parameter("xt", [128, ND * S], BF16, isOutput=False)
    xtkv = nc.declare_dram_parameter("xtkv", [128, ND * skv], BF16, isOutput=False)
    wq = nc.declare_dram_parameter("wq", [128, ND * NCOL], BF16, isOutput=False)
    wk = nc.declare_dram_parameter("wk", [128, ND * NCOL], BF16, isOutput=False)
    wv = nc.declare_dram_parameter("wv", [128, ND * NCOL], BF16, isOutput=False)
    wo = nc.declare_dram_parameter("wo", [128, NC2 * D], BF16, isOutput=False)
    bq = nc.declare_dram_parameter("bq", [NCOL], F32, isOutput=False)
    bk = nc.declare_dram_parameter("bk", [NCOL], F32, isOutput=False)
    bv = nc.declare_dram_parameter("bv", [NCOL], F32, isOutput=False)
    ew = nc.declare_dram_parameter("ew", [NCOL], F32, isOutput=False)
    maskb = nc.declare_dram_parameter("maskb", [skv], F32, isOutput=False)
    out = nc.declare_dram_parameter("out", [S, D], BF16, isOutput=True)

    with tile.TileContext(nc) as tc:
        with tc.tile_pool(name="singles", bufs=1) as singles:
            # --- persistent SBUF tiles -----------------------------------
            twqa = singles.tile([128, ND * NCOL], BF16, tag="wqa", name="twqa")
            twka = singles.tile([128, ND * NCOL], BF16, tag="wka", name="twka")
            twva = singles.tile([128, ND * NCOL], BF16, tag="wva", name="twva")
            twoa = singles.tile([128, NC2 * D], BF16, tag="woa", name="twoa")
            txta = singles.tile([128, ND * S], BF16, tag="xta", name="txta")
            txkva = singles.tile([128, ND * skv], BF16, tag="xkva", name="txkva")
            twq = [twqa[:, d * NCOL:(d + 1) * NCOL] for d in range(ND)]
            twk = [twka[:, d * NCOL:(d + 1) * NCOL] for d in range(ND)]
            twv = [twva[:, d * NCOL:(d + 1) * NCOL] for d in range(ND)]
            two = [twoa[:, c * D:(c + 1) * D] for c in range(NC2)]
            txt = [txta[:, d * S:(d + 1) * S] for d in range(ND)]
            txkv = [txkva[:, d * skv:(d + 1) * skv] for d in range(ND)]
            tqt = [singles.tile([128, S], BF16, tag=f"qt{c}", name=f"qt{c}") for c in range(NC2)]
            tkt = [singles.tile([128, skv], BF16, tag=f"kt{c}", name=f"kt{c}") for c in range(NC2)]
            tv = [
                [singles.tile([128, DH + 1], BF16, tag=f"v{h}_{j}", name=f"v{h}_{j}") for j in range(nj)]
                for h in range(HPC)
            ]
            tot = [singles.tile([128, S], F32, tag=f"ot{c}", name=f"ot{c}") for c in range(NC2)]
            totn = [singles.tile([128, S], BF16, tag=f"otn{c}", name=f"otn{c}") for c in range(NC2)]
            # softmax denominators: rows 0/32/64/96 hold heads 0..3
            tstage = singles.tile([97, S], F32, tag="lstage", name="tstage")
            trecf = singles.tile([97, S], F32, tag="lrecf", name="trecf")
            trec = singles.tile([97, S], BF16, tag="lrec", name="trec")
            tones4 = singles.tile([97, 64], BF16, tag="ones4", name="tones4")
            tonesf = singles.tile([128, 64], F32, tag="onesf", name="tonesf")
            tmb = [singles.tile([128, 1], F32, tag=f"mb{j}", name=f"mb{j}") for j in range(nj)]
            tbiasq = [singles.tile([128, 1], F32, tag=f"bq{c}", name=f"bq{c}") for c in range(NC2)]
            tbiask = [singles.tile([128, 1], F32, tag=f"bk{c}", name=f"bkt{c}") for c in range(NC2)]
            tbq_raw = [singles.tile([128, 1], F32, tag=f"bqr{c}", name=f"bqr{c}") for c in range(NC2)]
            tew_raw = [singles.tile([128, 1], F32, tag=f"ewr{c}", name=f"ewr{c}") for c in range(NC2)]
            tbvb = singles.tile([128, NCOL], F32, tag="bvb", name="bvb")

            # --- input DMAs ----------------------------------------------
            # sync queue: first half of the xt stream (Q proj) then the
            # phase-2 out stores.  scalar queue (HWDGE, engine idle in
            # phase 1): wq halves first (Q proj critical path), then the
            # other xt half, then xtkv (K/V proj), then wo.
            # gpsimd queue: small tiles, then wk, wv.
            half = ND // 2
            nc.scalar.dma_start(
                out=twqa[:, : half * NCOL], in_=wq[:, : half * NCOL]
            )
            nc.scalar.dma_start(
                out=twqa[:, half * NCOL:], in_=wq[:, half * NCOL:]
            )
            for d in range(half):
                nc.sync.dma_start(out=txt[d], in_=xt[:, d * S:(d + 1) * S])
            for d in range(half, ND):
                nc.scalar.dma_start(out=txt[d], in_=xt[:, d * S:(d + 1) * S])
            for d in range(ND):
                nc.scalar.dma_start(
                    out=txkv[d], in_=xtkv[:, d * skv:(d + 1) * skv]
                )
            nc.scalar.dma_start(out=twoa, in_=wo[:, :])
            for c in range(NC2):
                nc.gpsimd.dma_start(
                    out=tbq_raw[c], in_=bq[c * 128:(c + 1) * 128].unsqueeze(1)
                )
                nc.gpsimd.dma_start(
                    out=tew_raw[c], in_=ew[c * 128:(c + 1) * 128].unsqueeze(1)
                )
                nc.gpsimd.dma_start(
                    out=tbiask[c], in_=bk[c * 128:(c + 1) * 128].unsqueeze(1)
                )
                nc.vector.tensor_add(out=tbiasq[c], in0=tbq_raw[c], in1=tew_raw[c])
            for j in range(nj):
                nc.gpsimd.dma_start(
                    out=tmb[j], in_=maskb[j * 128:(j + 1) * 128].unsqueeze(1)
                )
            bvap = bv.ap()
            bv_bcast = bass.AP(
                tensor=bvap.tensor, offset=bvap.offset, ap=[[0, 128]] + list(bvap.ap)
            )
            nc.gpsimd.dma_start(out=tbvb, in_=bv_bcast)
            nc.gpsimd.dma_start(out=twka, in_=wk[:, :])
            nc.gpsimd.dma_start(out=twva, in_=wv[:, :])
            # memset cannot write non-fp32, so round ones through the DVE.
            nc.vector.memset(tonesf, 1.0)
            for h in range(HPC):
                nc.vector.tensor_copy(
                    out=tones4[32 * h:32 * h + 1, :], in_=tonesf[0:1, :]
                )

            # --- phase 1: projections (Q, K, V) --------------------------
            with (
                tc.tile_pool(name="pproj", bufs=4, space="PSUM") as pproj,
                tc.tile_pool(name="pv", bufs=2, space="PSUM") as pv,
            ):
                # ones column of V
                for h in range(HPC):
                    for j in range(nj):
                        nc.vector.tensor_copy(
                            out=tv[h][j][:, DH:DH + 1], in_=tonesf[:, 0:1]
                        )
                # QT[n, i] accumulated over d
                for i in range(NI):
                    for c in range(NC2):
                        ps = pproj.tile([128, 512], F32, tag="pp", name="pp")
                        for d in range(ND):
                            nc.tensor.matmul(
                                ps,
                                twq[d][:, c * 128:(c + 1) * 128],
                                txt[d][:, i * 512:(i + 1) * 512],
                                start=(d == 0),
                                stop=(d == ND - 1),
                            )
                        nc.vector.tensor_scalar_add(
                            out=tqt[c][:, i * 512:(i + 1) * 512],
                            in0=ps,
                            scalar1=tbiasq[c],
                        )
                # KT[n, j] accumulated over d
                for (jo, jw) in _chunks(skv, 512):
                    for c in range(NC2):
                        ps = pproj.tile([128, 512], F32, tag="pp", name="pp")
                        for d in range(ND):
                            nc.tensor.matmul(
                                ps[:, 0:jw],
                                twk[d][:, c * 128:(c + 1) * 128],
                                txkv[d][:, jo:jo + jw],
                                start=(d == 0),
                                stop=(d == ND - 1),
                            )
                        nc.vector.tensor_scalar_add(
                            out=tkt[c][:, jo:jo + jw],
                            in0=ps[:, 0:jw],
                            scalar1=tbiask[c],
                        )
                # V[j, n] accumulated over d, split per head (+bias bv)
                for j in range(nj):
                    ps = pv.tile([128, NCOL], F32, tag="pv", name="pvt")
                    for d in range(ND):
                        nc.tensor.matmul(
                            ps,
                            txkv[d][:, j * 128:(j + 1) * 128],
                            twv[d],
                            start=(d == 0),
                            stop=(d == ND - 1),
                        )
                    for h in range(HPC):
                        nc.vector.tensor_add(
                            out=tv[h][j][:, 0:DH],
                            in0=ps[:, h * DH:(h + 1) * DH],
                            in1=tbvb[:, h * DH:(h + 1) * DH],
                        )

            # --- phase 2: attention + normalize + output projection ------
            # j-loop software-pipelined: attnV(j-1) is emitted after exp(j),
            # so the in-order PE never head-of-line blocks on the exp; each
            # block's normalize + final-projection matmuls are deferred into
            # later j-loops via the pending queue to fill PE bubbles.
            with (
                tc.tile_pool(name="pts", bufs=3) as pts,
                tc.tile_pool(name="obuf", bufs=4) as obuf,
                tc.tile_pool(name="ps2", bufs=2, space="PSUM") as ps2,
                tc.tile_pool(name="pot", bufs=2, space="PSUM") as pot,
                tc.tile_pool(name="plf", bufs=2, space="PSUM") as plf,
            ):
                pending = []

                def emit_norm(i):
                    # normalize both c-chunks of block i: broadcast 1/l
                    # across the DH partitions via the ones matmul, then
                    # write the bf16 normalized OT for the out projection.
                    isl = slice(i * 512, (i + 1) * 512)
                    for c in range(NC2):
                        hA, hB = 2 * c, 2 * c + 1
                        plA = plf.tile([64, 512], F32, tag="plf", name="plA")
                        plB = plf.tile([64, 512], F32, tag="plf", name="plB")
                        nc.tensor.matmul(
                            plA,
                            tones4[32 * hA:32 * hA + 1, :],
                            trec[32 * hA:32 * hA + 1, isl],
                            start=True,
                            stop=True,
                            tile_position=(32 * hA, 0),
                        )
                        nc.tensor.matmul(
                            plB,
                            tones4[32 * hB:32 * hB + 1, :],
                            trec[32 * hB:32 * hB + 1, isl],
                            start=True,
                            stop=True,
                            tile_position=(32 * hB, 0),
                        )
                        nc.vector.tensor_mul(
                            out=totn[c][0:64, isl], in0=tot[c][0:64, isl], in1=plA
                        )
                        nc.vector.tensor_mul(
                            out=totn[c][64:128, isl], in0=tot[c][64:128, isl], in1=plB
                        )

                def emit_pf(i, so):
                    sidx = i * 4 + so
                    ssl = slice(sidx * 128, (sidx + 1) * 128)
                    for n in range(2):
                        nsl = slice(n * 512, (n + 1) * 512)
                        pf = plf.tile([128, 512], F32, tag="plf", name="pft")
                        for c in range(NC2):
                            nc.tensor.matmul(
                                pf,
                                totn[c][:, ssl],
                                two[c][:, nsl],
                                start=(c == 0),
                                stop=(c == NC2 - 1),
                            )
                        ob = obuf.tile([128, 512], BF16, tag="ob", name="obt")
                        nc.vector.tensor_copy(out=ob, in_=pf)
                        nc.sync.dma_start(out=out[ssl, nsl], in_=ob)

                for i in range(NI):
                    isl = slice(i * 512, (i + 1) * 512)
                    for c in range(NC2):
                        hA, hB = 2 * c, 2 * c + 1
                        potA = pot.tile([DH + 1, 512], F32, tag="pot", name="pott")
                        potB = pot.tile([DH + 1, 512], F32, tag="pot", name="pott")
                        pts_hist = []
                        for j in range(nj):
                            pscore = ps2.tile([128, 1024], F32, tag="ps", name="pscore")
                            nc.tensor.matmul(
                                pscore[:, 0:512],
                                tkt[c][0:64, j * 128:(j + 1) * 128],
                                tqt[c][0:64, isl],
                                start=True,
                                stop=True,
                                tile_position=(0, 0),
                            )
                            nc.tensor.matmul(
                                pscore[:, 512:1024],
                                tkt[c][64:128, j * 128:(j + 1) * 128],
                                tqt[c][64:128, isl],
                                start=True,
                                stop=True,
                                tile_position=(64, 0),
                            )
                            pt = pts.tile([128, 1024], BF16, tag="pt", name="ptile")
                            nc.scalar.activation(
                                out=pt,
                                in_=pscore,
                                func=AF.Exp,
                                bias=tmb[j],
                                scale=1.0 / math.sqrt(DH),
                            )
                            pts_hist.append(pt)
                            if j > 0:
                                pprev = pts_hist[j - 1]
                                nc.tensor.matmul(
                                    potA, tv[hA][j - 1], pprev[:, 0:512],
                                    start=(j - 1 == 0), stop=False,
                                )
                                nc.tensor.matmul(
                                    potB, tv[hB][j - 1], pprev[:, 512:1024],
                                    start=(j - 1 == 0), stop=False,
                                )
                            if pending and (
                                j % 2 == 1 or len(pending) > 7 or i == NI - 1
                            ):
                                pending.pop(0)()
                        nc.tensor.matmul(
                            potA, tv[hA][nj - 1], pts_hist[nj - 1][:, 0:512],
                            start=(nj == 1), stop=True,
                        )
                        nc.tensor.matmul(
                            potB, tv[hB][nj - 1], pts_hist[nj - 1][:, 512:1024],
                            start=(nj == 1), stop=True,
                        )
                        nc.vector.tensor_copy(out=tot[c][0:64, isl], in_=potA[0:DH, :])
                        nc.vector.tensor_copy(out=tot[c][64:128, isl], in_=potB[0:DH, :])
                        nc.vector.tensor_copy(
                            out=tstage[32 * hA:32 * hA + 1, isl],
                            in_=potA[DH:DH + 1, :],
                        )
                        nc.vector.tensor_copy(
                            out=tstage[32 * hB:32 * hB + 1, isl],
                            in_=potB[DH:DH + 1, :],
                        )
                    # batched softmax-denominator reciprocal for this block;
                    # the normalize matmuls + muls are deferred into later
                    # j-loops so the PE never waits on this DVE chain.
                    nc.vector.reciprocal_approx_fast(
                        out=trecf[:, isl], in_=tstage[:, isl]
                    )
                    nc.vector.tensor_copy(out=trec[:, isl], in_=trecf[:, isl])
                    pending.append(lambda i=i: emit_norm(i))
                    for so in range(4):
                        pending.append(lambda i=i, so=so: emit_pf(i, so))
                while pending:
                    pending.pop(0)()

    nc.compile()
    return nc


def _get_program(skv):
    if skv not in _PROGRAM_CACHE:
        _PROGRAM_CACHE[skv] = build_program(skv)
    return _PROGRAM_CACHE[skv]


def _pack_chunks(a, nchunk):
    """[nchunk*128, C] -> [128, nchunk*C] chunk-major (4KB-line DMAs)."""
    c = a.shape[1]
    return np.ascontiguousarray(
        a.reshape(nchunk, 128, c).transpose(1, 0, 2).reshape(128, nchunk * c)
    )


def _shard_inputs(hidden_states, attention_mask, Wq, bq, Wk, bk, Wv, bv,
                  emotion_w, Wo, bo):
    hs = np.asarray(hidden_states, dtype=np.float32)
    mask = np.asarray(attention_mask)
    Wq = np.asarray(Wq, dtype=np.float32)
    Wk = np.asarray(Wk, dtype=np.float32)
    Wv = np.asarray(Wv, dtype=np.float32)
    Wo = np.asarray(Wo, dtype=np.float32)
    bq = np.asarray(bq, dtype=np.float32)
    bk = np.asarray(bk, dtype=np.float32)
    bv = np.asarray(bv, dtype=np.float32)
    ew = np.asarray(emotion_w, dtype=np.float32)

    idx = [np.nonzero(mask[b])[0] for b in range(B)]
    sv = max(len(ix) for ix in idx)
    skv = max(128, ((sv + 127) // 128) * 128)

    in_maps = []
    for b in range(B):
        xt_b = _pack_chunks(hs[b].T.astype(BF16NP), ND)  # [128, ND*S]
        xtkv_f = np.zeros((D, skv), dtype=BF16NP)
        xtkv_f[:, : len(idx[b])] = hs[b][idx[b]].T.astype(BF16NP)
        xtkv_b = _pack_chunks(xtkv_f, ND)
        maskb_b = np.zeros(skv, dtype=np.float32)
        maskb_b[len(idx[b]):] = -1e30
        for hg in range(H // HPC):
            cols = slice(hg * NCOL, (hg + 1) * NCOL)
            in_maps.append(
                {
                    "xt": xt_b,
                    "xtkv": xtkv_b,
                    "wq": _pack_chunks(Wq[:, cols].astype(BF16NP), ND),
                    "wk": _pack_chunks(Wk[:, cols].astype(BF16NP), ND),
                    "wv": _pack_chunks(Wv[:, cols].astype(BF16NP), ND),
                    "wo": _pack_chunks(Wo[cols, :].astype(BF16NP), NC2),
                    "bq": np.ascontiguousarray(bq[cols]),
                    "bk": np.ascontiguousarray(bk[cols]),
                    "bv": np.ascontiguousarray(bv[cols]),
                    "ew": np.ascontiguousarray(
                        ew[hg * HPC:(hg + 1) * HPC].reshape(NCOL)
                    ),
                    "maskb": maskb_b,
                }
            )
    return in_maps, skv, np.asarray(bo, dtype=np.float32)


def run(inputs, trace=False, trace_kwargs=None):
    in_maps, skv, bo = _shard_inputs(**inputs)
    nc = _get_program(skv)
    res = run_bass_kernel_spmd(
        nc,
        in_maps,
        core_ids=list(range(8)),
        trace=trace,
        **(trace_kwargs or {}),
    )
    out = np.zeros((B, S, D), dtype=np.float32)
    for b in range(B):
        acc = np.zeros((S, D), dtype=np.float64)
        for hg in range(4):
            acc += np.asarray(res.results[b * 4 + hg]["out"], dtype=np.float32)
        out[b] = (acc + bo).astype(np.float32)
    return out, res


def kernel(**inputs):
    out, _ = run(inputs, trace=False)
    return out
